# revision 63
# baseline (speedup 1.0000x reference)
"""Trainium2 Bass kernel for nn_MemTransformerLM (hourglass Transformer-XL).

Sharding: 8 cores = 4 batch rows x 2-way "Megatron-SP" pairs.

Full-length stages (layers 0,1,6,7): attention is tensor-parallel over heads
(4/core, structurally identical across cores); the o-projection partial is
ReduceScattered over token halves (f32), LN + full-d_inner FFN run on the own
512-token half, and the post-LN2 activations are AllGathered (bf16) when the
next layer needs full-length k/v. Layer 0 folds the embedding residual into
the ReduceScatter (each core adds 0.5x the shared embedding to its partial).

The shortened stage (257 segments padded to 384) is cheap, so both cores of a
pair compute it fully replicated with all 8 heads -- its only collective is
one AllGather of the pooled-segment slabs (each core pools its own token
half, whose segments are a contiguous half of the segment axis).

Collective bytes per pair drop ~4x vs the per-layer AllReduce scheme and the
short stage needs no collectives at all.

Activations flow transposed ("T-layout": model dim on partitions, tokens on
the free axis). Weights are host-pre-transposed to contraction-major bf16.
The Transformer-XL rel_shift runs on GPSIMD local_scatter (per-partition
staircase indices; negative index = causal drop). Softmax skips the max
subtraction (scores provably small); denominators fall out of the Exp
activation's accum_out during PSUM eviction.
"""
import os
import sys
sys.path.insert(0, '/opt/trn_rl_repo')

import numpy as np
import ml_dtypes

import concourse.bass as bass
import concourse.tile as tile
from concourse import bacc, mybir
from concourse.bass_utils import run_bass_kernel_spmd

F32 = mybir.dt.float32
BF16 = mybir.dt.bfloat16
F8 = mybir.dt.float8e4
I16 = mybir.dt.int16
AF = mybir.ActivationFunctionType
ALU = mybir.AluOpType
DR = mybir.MatmulPerfMode.DoubleRow
W8S = 32.0        # fp8 weight pre-scale (undone by activation scale=1/32)

T, B, D, H, DH, DI, V, L = 1024, 4, 512, 8, 64, 2048, 256, 8
S = 256
SP = 384          # padded short length (3 tiles; real rows 0..256)
KD = D // 128     # 4 d-tiles
NKI = DI // 128   # 16 ff-inner k-tiles (full d_inner per core)
NEG = -1.0e30
SCALE = 0.125
GROUPS = [[0, 1], [2, 3], [4, 5], [6, 7]]
FULL_LAYERS = (0, 1, 6, 7)
SHORT_LAYERS = (2, 3, 4, 5)

N_CORES = 8
REPS = int(os.environ.get('KERNEL_REPS', '1'))
NO_COLL = os.environ.get('KERNEL_NO_COLL') == '1'


def ts(i, n=128):
    return slice(i * n, (i + 1) * n)


def chunk_list(NT):
    return [(c * 512, min(512, NT - c * 512)) for c in range((NT + 511) // 512)]


def _ln(nc, p, psum, xpre, NT, g, b, fp8=False):
    """LayerNorm over the partition (d) axis in T-layout.
    Stats via ones-matmuls; returns (XF fp32 tiles, XB bf16 tiles), or with
    fp8=True (XF, XQ8) where XQ8 is 2 fp8 tiles [128, 2, NT] pairing d-tiles
    for DoubleRow matmuls."""
    chunks = chunk_list(NT)
    ones_b = _ln.ones_b
    ones_row = _ln.ones_row
    arow = p.tile([1, NT], F32, tag="arow")
    brow = p.tile([1, NT], F32, tag="brow")
    for c0, cw in chunks:
        ps1 = psum.tile([1, 512], F32, tag="st1")[:, :cw]
        ps2 = psum.tile([1, 512], F32, tag="st2")[:, :cw]
        for m in range(KD):
            xbf = p.tile([128, 512], BF16, tag="xbf")[:, :cw]
            nc.vector.tensor_copy(xbf, xpre[m][:, c0:c0 + cw])
            nc.tensor.matmul(ps1, ones_b, xbf, start=(m == 0), stop=(m == KD - 1))
            sqm = p.tile([128, 512], BF16, tag="sq")[:, :cw]
            nc.gpsimd.tensor_tensor(sqm, xpre[m][:, c0:c0 + cw],
                                    xpre[m][:, c0:c0 + cw], ALU.mult)
            nc.tensor.matmul(ps2, ones_b, sqm, start=(m == 0), stop=(m == KD - 1))
        mean = p.tile([1, 512], F32, tag="mean")[:, :cw]
        var = p.tile([1, 512], F32, tag="var")[:, :cw]
        msq = p.tile([1, 512], F32, tag="msq")[:, :cw]
        nc.vector.tensor_scalar_mul(mean, ps1, 1.0 / D)
        nc.vector.tensor_scalar_mul(var, ps2, 1.0 / D)
        nc.vector.tensor_tensor(msq, mean, mean, ALU.mult)
        nc.vector.tensor_tensor(var, var, msq, ALU.subtract)
        nc.vector.tensor_scalar_add(var, var, 1.0e-5)
        nc.vector.reciprocal(var, var)
        nc.scalar.activation(arow[:, c0:c0 + cw], var, AF.Sqrt)
        nc.vector.tensor_tensor(brow[:, c0:c0 + cw], mean, arow[:, c0:c0 + cw], ALU.mult)
        nc.vector.tensor_scalar_mul(brow[:, c0:c0 + cw], brow[:, c0:c0 + cw], -1.0)
    XF, XB = [], []
    XQ8 = [p.tile([128, 2, NT], F8, tag="xq8", bufs=4) for _ in range(2)] if fp8 else None
    for m in range(KD):
        xf = p.tile([128, NT], F32, tag="XF", name=f"XF{m}", bufs=4)
        for c0, cw in chunks:
            aps = psum.tile([128, 512], F32, tag="sc")[:, :cw]
            bps = psum.tile([128, 512], F32, tag="sc")[:, :cw]
            nc.tensor.matmul(aps, ones_row, arow[:, c0:c0 + cw], start=True, stop=True)
            nc.tensor.matmul(bps, ones_row, brow[:, c0:c0 + cw], start=True, stop=True)
            t1 = p.tile([128, 512], F32, tag="lnt")[:, :cw]
            nc.vector.tensor_tensor(t1, xpre[m][:, c0:c0 + cw], aps, ALU.mult)
            nc.vector.tensor_tensor(t1, t1, bps, ALU.add)
            nc.scalar.activation(xf[:, c0:c0 + cw], t1, AF.Identity, bias=b[:, m:m + 1],
                                 scale=g[:, m:m + 1])
        if fp8:
            nc.vector.tensor_copy(XQ8[m // 2][:, m % 2, :], xf[:])
        else:
            xb = p.tile([128, NT], BF16, tag="XB", name=f"XB{m}", bufs=4)
            nc.vector.tensor_copy(xb[:], xf[:])
            XB.append(xb)
        XF.append(xf)
    return XF, (XQ8 if fp8 else XB)


def _ag_x(nc, p, dram, xb, NTK, tag):
    """AllGather own-half bf16 activations into full-length tiles.
    Rank slabs are contiguous token halves, so bout is already in true
    token order."""
    HN = NTK // 2
    bin_ = dram.tile([KD, 128, HN], BF16, tag=f"agi_{tag}")
    bout = dram.tile([2, KD, 128, HN], BF16, tag=f"ago_{tag}")
    for m in range(KD):
        nc.sync.dma_start(bin_[m], xb[m][:])
    if NO_COLL:
        nc.sync.dma_start(bout[0], bin_[:])
        nc.sync.dma_start(bout[1], bin_[:])
    else:
        nc.gpsimd.collective_compute(
            "AllGather", ALU.bypass, replica_groups=GROUPS,
            ins=[bin_.opt()], outs=[bout.opt()])
    XBn = []
    for m in range(KD):
        xa = p.tile([128, NTK], BF16, tag="XBall", name=f"XBall{m}")
        nc.scalar.dma_start(xa[:, 0:HN], bout[0, m])
        nc.scalar.dma_start(xa[:, HN:NTK], bout[1, m])
        XBn.append(xa)
    return XBn


def _layer(nc, pools, lw, XF, XBall, NTK, NTF, consts, do_rs, do_ag, fold=None):
    """One transformer layer.

    XBall: KD bf16 tiles [128, NTK] -- all k/v tokens (true order).
    XF:    KD f32 tiles [128, NTF] -- residual for the ffn-token set
           (own half for full stages, everything for the short stage).
           None only for layer 0 (residual folded into the RS via `fold`).
    do_rs: ReduceScatter the o-projection over token halves (full stages).
    do_ag: AllGather LN2 output into new XBall tiles for the next layer.
    """
    p, psum, dram = pools
    HT = lw['HT']                       # head-tiles (2 heads each)
    chunks = chunk_list(NTK)
    idbf, sinT = consts['idbf'], consts['sinT']
    rwb, rrb = lw['rwb'], lw['rrb']
    QC = HT * 128
    vcols = slice(2 * HT * 128, 3 * HT * 128)
    sfx = "s" if HT == 4 else ""        # short-stage tiles get their own tags
    qbufs = 4 if HT == 4 else 2

    # --- uT: rk projection against the position-sinusoid table ---
    uT = []
    for m in range(HT):
        u = p.tile([128, NTK], BF16, tag="uT" + sfx, bufs=qbufs)
        for c0, cw in chunks:
            ps = psum.tile([128, 512], F32, tag="sc")[:, :cw]
            for kd in range(KD):
                nc.tensor.matmul(ps, lw['wrkT'][kd][:, ts(m)], sinT[kd][:, c0:c0 + cw],
                                 start=(kd == 0), stop=(kd == KD - 1))
            nc.scalar.copy(u[:, c0:c0 + cw], ps)
        uT.append(u)

    # --- q/k in T-layout (all tokens) ---
    qac, qbd, kb = [], [], []
    for m in range(HT):
        qa = p.tile([128, NTK], BF16, tag="qac" + sfx, bufs=qbufs)
        qb = p.tile([128, NTK], BF16, tag="qbd" + sfx, bufs=qbufs)
        kk = p.tile([128, NTK], BF16, tag="kb" + sfx, bufs=qbufs)
        for c0, cw in chunks:
            ps = psum.tile([128, 512], F32, tag="sc")[:, :cw]
            for kd in range(KD):
                nc.tensor.matmul(ps, lw['wqkvT'][kd][:, ts(m)], XBall[kd][:, c0:c0 + cw],
                                 start=(kd == 0), stop=(kd == KD - 1))
            nc.scalar.activation(qa[:, c0:c0 + cw], ps, AF.Identity, bias=rwb[:, m:m + 1])
            nc.vector.tensor_scalar_add(qb[:, c0:c0 + cw], ps, rrb[:, m:m + 1])
            ps2 = psum.tile([128, 512], F32, tag="sc")[:, :cw]
            for kd in range(KD):
                nc.tensor.matmul(ps2, lw['wqkvT'][kd][:, ts(HT + m)], XBall[kd][:, c0:c0 + cw],
                                 start=(kd == 0), stop=(kd == KD - 1))
            nc.scalar.copy(kk[:, c0:c0 + cw], ps2)
        qac.append(qa)
        qbd.append(qb)
        kb.append(kk)

    # --- v in N-layout (all tokens) ---
    vb = []
    for tt in range(NTK // 128):
        v = p.tile([128, QC], BF16, tag="vb", name=f"vb{tt}", bufs=8)
        ps = psum.tile([128, 512], F32, tag="sc")[:, :QC]
        for kd in range(KD):
            nc.tensor.matmul(ps, XBall[kd][:, ts(tt)], lw['wqkvT'][kd][:, vcols],
                             start=(kd == 0), stop=(kd == KD - 1))
        nc.vector.tensor_copy(v[:], ps)
        vb.append(v)

    # --- attention: qi outer, head-group / head inner ---
    voT = [p.tile([128, NTK], BF16, tag="voT" + sfx, name=f"voT{m}", bufs=qbufs)
           for m in range(HT)]
    for qi in range(NTK // 128):
        W = (qi + 1) * 128
        i0 = qi * 128
        idx = p.tile([128, NTK], I16, tag="idx")
        nc.gpsimd.iota(idx[:, :W], pattern=[[-1, W]], base=i0, channel_multiplier=1)
        wch = [(c0, min(cw, W - c0)) for c0, cw in chunks if c0 < W]
        for g in range(HT // 2):
            pvall = psum.tile([64, 4, 128], F32, tag="pv")
            den_mat = p.tile([128, 4], F32, tag="den")
            for hh in range(4):
                head = g * 4 + hh
                mi, po = head // 2, (head % 2) * 64
                bdd = p.tile([128, NTK], BF16, tag="bdd", bufs=2)
                for c0, cw in wch:
                    ps = psum.tile([128, 512], F32, tag="bd")[:, :cw]
                    nc.tensor.matmul(ps, qbd[mi][po:po + 64, i0:i0 + 128],
                                     uT[mi][po:po + 64, c0:c0 + cw], start=True, stop=True)
                    nc.scalar.copy(bdd[:, c0:c0 + cw], ps)
                bds = p.tile([128, NTK], BF16, tag="bds", bufs=2)
                nc.gpsimd.local_scatter(bds[:, :W], bdd[:, :W], idx[:, :W],
                                        channels=128, num_elems=W, num_idxs=W)
                nc.gpsimd.affine_select(bds[:, i0:W], bds[:, i0:W], pattern=[[-1, 128]],
                                        compare_op=ALU.is_ge, fill=NEG,
                                        base=0, channel_multiplier=1)
                ex = p.tile([128, NTK], BF16, tag="ex", bufs=3)
                den2 = p.tile([128, 2], F32, tag="den2")
                for ci, (c0, cw) in enumerate(wch):
                    ps = psum.tile([128, 512], F32, tag="bd")[:, :cw]
                    nc.tensor.matmul(ps, idbf, bds[:, c0:c0 + cw], start=True, stop=False)
                    nc.tensor.matmul(ps, qac[mi][po:po + 64, i0:i0 + 128],
                                     kb[mi][po:po + 64, c0:c0 + cw], start=False, stop=True)
                    nc.scalar.activation(ex[:, c0:c0 + cw], ps, AF.Exp, scale=SCALE,
                                         accum_out=den2[:, ci:ci + 1])
                if len(wch) == 1:
                    nc.vector.reciprocal(den_mat[:, hh:hh + 1], den2[:, 0:1])
                else:
                    nc.vector.tensor_tensor(den_mat[:, hh:hh + 1], den2[:, 0:1],
                                            den2[:, 1:2], ALU.add)
                    nc.vector.reciprocal(den_mat[:, hh:hh + 1], den_mat[:, hh:hh + 1])
                nc.vector.tensor_scalar_mul(ex[:, :W], ex[:, :W], den_mat[:, hh:hh + 1])
                for jt in range(qi + 1):
                    ext = p.tile([128, 128], BF16, tag="ext")
                    ept = psum.tile([128, 128], BF16,
                                    tag=("st1" if jt % 2 == 0 else "st2"))
                    nc.tensor.transpose(ept[:], ex[:, ts(jt)], idbf)
                    if head % 2 == 0:
                        nc.scalar.copy(ext[:], ept[:])
                    else:
                        nc.vector.tensor_copy(ext[:], ept[:])
                    nc.tensor.matmul(pvall[:, hh], vb[jt][:, head * 64:head * 64 + 64],
                                     ext[:], start=(jt == 0), stop=(jt == qi))
            for hh in range(4):
                head = g * 4 + hh
                mi, po = head // 2, (head % 2) * 64
                nc.vector.tensor_copy(voT[mi][po:po + 64, i0:i0 + 128], pvall[:, hh])

    # --- o-proj; ReduceScatter over token halves (full) or direct (short) ---
    li, rep = lw['li'], lw['rep']
    if do_rs:
        bin_ = dram.tile([2, KD, 128, NTF], BF16, tag=f"rsi_{li}_{rep}")
        bout = dram.tile([KD, 128, NTF], BF16, tag=f"rso_{li}_{rep}")
        for m in range(KD):
            for ci, (c0, cw) in enumerate(chunks):
                ps = psum.tile([128, 512], F32, tag="sc")[:, :cw]
                for kt in range(HT):
                    nc.tensor.matmul(ps, lw['woT'][kt][:, ts(m)], voT[kt][:, c0:c0 + cw],
                                     start=(kt == 0), stop=(kt == HT - 1))
                ob = p.tile([128, 512], BF16, tag="obh")[:, :cw]
                if m % 2 == 0:
                    nc.scalar.copy(ob, ps)
                else:
                    nc.vector.tensor_copy(ob, ps)
                nc.sync.dma_start(bin_[ci, m], ob)
        if NO_COLL:
            nc.sync.dma_start(bout[:], bin_[0])
        else:
            nc.gpsimd.collective_compute(
                "ReduceScatter", ALU.add, replica_groups=GROUPS,
                ins=[bin_.opt()], outs=[bout.opt()])
        xpre = []
        for m in range(KD):
            xr = p.tile([128, NTF], BF16, tag="arout")
            nc.scalar.dma_start(xr[:], bout[m])
            xp = p.tile([128, NTF], F32, tag="big", name=f"xp{m}")
            if XF is None:
                nc.vector.tensor_copy(xp[:], xr[:])
            else:
                nc.vector.tensor_tensor(xp[:], xr[:], XF[m][:], ALU.add)
            xpre.append(xp)
    else:
        xpre = []
        for m in range(KD):
            xp = p.tile([128, NTF], F32, tag="big", name=f"xp{m}")
            for c0, cw in chunk_list(NTF):
                ps = psum.tile([128, 512], F32, tag="sc")[:, :cw]
                for kt in range(HT):
                    nc.tensor.matmul(ps, lw['woT'][kt][:, ts(m)], voT[kt][:, c0:c0 + cw],
                                     start=(kt == 0), stop=(kt == HT - 1))
                nc.vector.tensor_tensor(xp[:, c0:c0 + cw], ps, XF[m][:, c0:c0 + cw],
                                        ALU.add)
            xpre.append(xp)
    XF1, XB1 = _ln(nc, p, psum, xpre, NTF, lw['g1'], lw['bb1'])

    # --- FFN in two d_inner halves (second half's weights stream in while
    # the first half computes; SBUF holds only half the FFN weights) ---
    NH2 = NKI // 2
    partial = []
    w_next = None
    for dh in range(2):
        if dh == 0:
            w1h, w2h = lw['ffn_w0']
            w_next = lw['load_ffn_half'](1)
        else:
            w1h, w2h = w_next
        hb = []
        for mi in range(NH2):
            hbt = p.tile([128, NTF], BF16, tag="hb", name=f"hb{mi}", bufs=NH2)
            for c0, cw in chunk_list(NTF):
                ps = psum.tile([128, 512], F32, tag="sc")[:, :cw]
                for kd in range(KD):
                    nc.tensor.matmul(ps, w1h[kd][:, ts(mi)], XB1[kd][:, c0:c0 + cw],
                                     start=(kd == 0), stop=(kd == KD - 1))
                nc.scalar.activation(hbt[:, c0:c0 + cw], ps, AF.Relu,
                                     bias=lw['fb1'][:, dh * NH2 + mi:dh * NH2 + mi + 1])
            hb.append(hbt)
        for m in range(KD):
            for c0, cw in chunk_list(NTF):
                ps = psum.tile([128, 512], F32, tag="sc")[:, :cw]
                for kt in range(NH2):
                    nc.tensor.matmul(ps, w2h[kt][:, ts(m)], hb[kt][:, c0:c0 + cw],
                                     start=(kt == 0), stop=(kt == NH2 - 1))
                if dh == 0:
                    pt = p.tile([128, NTF], F32, tag="fpart", name=f"fp{m}", bufs=4)
                    nc.vector.tensor_copy(pt[:, c0:c0 + cw], ps)
                    if c0 == 0:
                        partial.append(pt)
                else:
                    t1 = p.tile([128, 512], F32, tag="lnt")[:, :cw]
                    nc.scalar.activation(t1, ps, AF.Identity,
                                         bias=lw['fb2'][:, m:m + 1])
                    nc.vector.tensor_tensor(partial[m][:, c0:c0 + cw],
                                            partial[m][:, c0:c0 + cw], t1, ALU.add)
    xpre2 = []
    for m in range(KD):
        xp = p.tile([128, NTF], F32, tag="big", name=f"xq{m}")
        nc.vector.tensor_tensor(xp[:], partial[m][:], XF1[m][:], ALU.add)
        xpre2.append(xp)
    XF2, XB2 = _ln(nc, p, psum, xpre2, NTF, lw['g2'], lw['bb2'])

    XBn = None
    if do_ag:
        XBn = _ag_x(nc, p, dram, XB2, NTK, tag=f"x{li}_{rep}")
    return XF2, XB2, XBn


def build_program():
    nc = bacc.Bacc("TRN2", target_bir_lowering=False, debug=False, num_devices=N_CORES)
    d = {}

    def di(name, shape, dt):
        d[name] = nc.dram_tensor(name, shape, dt, kind="ExternalInput")

    di("wqkvT4", [4, D, 768], BF16)
    di("wrkT4", [4, D, 256], BF16)
    di("woT4", [4, 256, D], BF16)
    di("wqkvS", [4, D, 1536], BF16)
    di("wrkS", [4, D, 512], BF16)
    di("woS", [4, 512, D], BF16)
    di("w1T", [L, D, DI], BF16)
    di("w2T", [L, DI, D], BF16)
    di("fb1", [L, DI], F32)
    di("fb2", [L, D], F32)
    di("g1", [L, D], F32)
    di("bb1", [L, D], F32)
    di("g2", [L, D], F32)
    di("bb2", [L, D], F32)
    di("rwb4", [256], F32)
    di("rrb4", [256], F32)
    di("rwbS", [512], F32)
    di("rrbS", [512], F32)
    di("wemb", [V, D], BF16)
    di("onehotT", [V, T], BF16)
    di("onehotTo", [V, T // 2], BF16)
    di("sinTd", [D, T], BF16)
    di("idbf", [128, 128], BF16)
    di("wpool", [4, 128, 128], BF16)
    di("nullv", [D], F32)
    di("gd", [D], F32)
    di("bdn", [D], F32)
    di("uupo", [SP, T // 2], BF16)
    di("finT", [D, V], BF16)
    di("fbn", [V], F32)
    logits = nc.dram_tensor("logits", [T // 2, V], F32, kind="ExternalOutput")

    with tile.TileContext(nc) as tc:
        import itertools
        _ctr = itertools.count()

        class NP:
            def __init__(self, pool):
                self.pool = pool

            def tile(self, shape, dt, tag=None, name=None, bufs=None):
                if name is None:
                    name = f"{tag}_{next(_ctr)}"
                return self.pool.tile(shape, dt, tag=tag, name=name, bufs=bufs)

        with tc.tile_pool(name="p", bufs=2) as p_r, \
             tc.tile_pool(name="pbig", bufs=5) as pbig_r, \
             tc.tile_pool(name="px", bufs=5) as px_r, \
             tc.tile_pool(name="pxa", bufs=4) as pxa_r, \
             tc.tile_pool(name="pres", bufs=4) as pres_r, \
             tc.tile_pool(name="pr", bufs=4) as pr_r, \
             tc.tile_pool(name="ph", bufs=8) as ph_r, \
             tc.tile_pool(name="pw", bufs=1) as pw_r, \
             tc.tile_pool(name="pwF", bufs=1) as pwF_r, \
             tc.tile_pool(name="pc", bufs=1) as pc_r, \
             tc.tile_pool(name="psum", bufs=2, space="PSUM") as psum_r, \
             tc.tile_pool(name="psum1", bufs=1, space="PSUM") as psum1_r, \
             tc.tile_pool(name="dram", bufs=1, space="DRAM") as dram_r:
            (p, pbig, px, pxa, pres, pr, ph, pw, pwF, pc) = (
                NP(x) for x in (p_r, pbig_r, px_r, pxa_r, pres_r, pr_r, ph_r,
                                pw_r, pwF_r, pc_r))
            psum_, psum1_, dram = NP(psum_r), NP(psum1_r), NP(dram_r)

            class P:
                def tile(self, shape, dt, tag=None, name=None, bufs=None):
                    if tag in ("XF", "XB"):
                        return px.tile(shape, dt, tag=tag, name=name, bufs=bufs)
                    if tag == "XBall":
                        return pxa.tile(shape, dt, tag=tag, name=name, bufs=bufs)
                    if tag in ("big",):
                        return pbig.tile(shape, dt, tag=tag, name=name, bufs=bufs)
                    if tag == "residF":
                        return pres.tile(shape, dt, tag=tag, name=name, bufs=bufs)
                    if tag in ("hb", "vb", "oh", "ohw", "xn"):
                        return ph.tile(shape, dt, tag=tag, name=name, bufs=bufs)
                    if tag in ("ext", "arout"):
                        return pr.tile(shape, dt, tag=tag, name=name, bufs=bufs)
                    if tag in ("idx", "arow", "msq"):
                        return pc.tile(shape, dt, tag=tag, name=name, bufs=bufs)
                    if tag is not None and tag.startswith("c_"):
                        return pc.tile(shape, dt, tag=tag, name=name, bufs=bufs)
                    return p.tile(shape, dt, tag=tag, name=name, bufs=bufs)
            pp = P()

            class PS:
                def tile(self, shape, dt, tag=None, name=None, bufs=None):
                    if tag in ("st1", "st2"):
                        return psum1_.tile(shape, dt, tag=tag, name=name)
                    return psum_.tile(shape, dt, tag=tag, name=name, bufs=bufs)
            pps = PS()
            pools = (pp, pps, dram)

            consts = {}
            idbf = pc.tile([128, 128], BF16, tag="c_idbf")
            nc.sync.dma_start(idbf[:], d["idbf"].ap())
            ones_b = pc.tile([128, 1], BF16, tag="c_ones")
            nc.gpsimd.memset(ones_b[:], 1.0)
            ones_row = pc.tile([1, 128], F32, tag="c_onesr")
            nc.gpsimd.memset(ones_row[:], 1.0)
            _ln.ones_b = ones_b
            _ln.ones_row = ones_row
            sinT = [pc.tile([128, T], BF16, tag=f"c_sin{k}") for k in range(KD)]
            for k in range(KD):
                nc.sync.dma_start(sinT[k][:], d["sinTd"].ap()[ts(k), :])
            rwb4 = pc.tile([128, 2], F32, tag="c_rwb4")
            nc.sync.dma_start(rwb4[:], d["rwb4"].ap().rearrange("(a q) -> q a", q=128))
            rrb4 = pc.tile([128, 2], F32, tag="c_rrb4")
            nc.sync.dma_start(rrb4[:], d["rrb4"].ap().rearrange("(a q) -> q a", q=128))
            rwbS = pc.tile([128, 4], F32, tag="c_rwbS")
            nc.sync.dma_start(rwbS[:], d["rwbS"].ap().rearrange("(a q) -> q a", q=128))
            rrbS = pc.tile([128, 4], F32, tag="c_rrbS")
            nc.sync.dma_start(rrbS[:], d["rrbS"].ap().rearrange("(a q) -> q a", q=128))
            consts.update(idbf=idbf, sinT=sinT)

            def load_biases(lw, li):
                for nm in ("fb1", "fb2", "g1", "bb1", "g2", "bb2"):
                    cols = NKI if nm == "fb1" else KD
                    tl = pw.tile([128, cols], F32, tag=f"w_{nm}")
                    nc.sync.dma_start(tl[:], d[nm].ap()[li].rearrange("(a q) -> q a", q=128))
                    lw[nm] = tl

            def load_ffn(lw, li):
                def load_half(dh):
                    w1h = [pwF.tile([128, DI // 2], BF16, tag=f"w_1{k}")
                           for k in range(KD)]
                    w2h = [pwF.tile([128, D], BF16, tag=f"w_2{k}")
                           for k in range(NKI // 2)]
                    for k in range(KD):
                        nc.sync.dma_start(
                            w1h[k][:],
                            d["w1T"].ap()[li, ts(k),
                                          dh * (DI // 2):(dh + 1) * (DI // 2)])
                    for k in range(NKI // 2):
                        nc.sync.dma_start(
                            w2h[k][:], d["w2T"].ap()[li, ts(dh * (NKI // 2) + k), :])
                    return w1h, w2h
                lw['load_ffn_half'] = load_half
                lw['ffn_w0'] = load_half(0)

            def load_layer_full(k4, li, rep):
                lw = {'li': li, 'rep': rep, 'HT': 2, 'rwb': rwb4, 'rrb': rrb4}
                lw['wqkvT'] = [pw.tile([128, 768], BF16, tag=f"w_qkv{k}")
                               for k in range(KD)]
                lw['wrkT'] = [pw.tile([128, 256], BF16, tag=f"w_rk{k}")
                              for k in range(KD)]
                lw['woT'] = [pw.tile([128, D], BF16, tag=f"w_o{k}")
                             for k in range(2)]
                for k in range(KD):
                    nc.sync.dma_start(lw['wqkvT'][k][:], d["wqkvT4"].ap()[k4, ts(k), :])
                    nc.sync.dma_start(lw['wrkT'][k][:], d["wrkT4"].ap()[k4, ts(k), :])
                for k in range(2):
                    nc.sync.dma_start(lw['woT'][k][:], d["woT4"].ap()[k4, ts(k), :])
                load_ffn(lw, li)
                load_biases(lw, li)
                return lw

            def load_layer_short(k4, li, rep):
                lw = {'li': li, 'rep': rep, 'HT': 4, 'rwb': rwbS, 'rrb': rrbS}
                lw['wqkvT'] = [pw.tile([128, 1536], BF16, tag=f"w_qkv{k}")
                               for k in range(KD)]
                lw['wrkT'] = [pw.tile([128, 512], BF16, tag=f"w_rk{k}")
                              for k in range(KD)]
                lw['woT'] = [pw.tile([128, D], BF16, tag=f"w_o{k}")
                             for k in range(4)]
                for k in range(KD):
                    nc.sync.dma_start(lw['wqkvT'][k][:], d["wqkvS"].ap()[k4, ts(k), :])
                    nc.sync.dma_start(lw['wrkT'][k][:], d["wrkS"].ap()[k4, ts(k), :])
                for k in range(4):
                    nc.sync.dma_start(lw['woT'][k][:], d["woS"].ap()[k4, ts(k), :])
                load_ffn(lw, li)
                load_biases(lw, li)
                return lw

            for rep in range(REPS):
                # --- embedding (one-hot matmul; full-T bf16 + own-half f32) ---
                XBall, x0own = [], []
                for m in range(KD):
                    xb = pp.tile([128, T], BF16, tag="XBall", name=f"XBall{m}")
                    xo = pp.tile([128, T // 2], F32, tag="big", name=f"x0o{m}")
                    wemb = pp.tile([128, 2, 128], BF16, tag="ohw", bufs=4)
                    nc.sync.dma_start(
                        wemb[:],
                        d["wemb"].ap().rearrange("(a q) e -> q a e", q=128)[:, :, ts(m)])
                    for c in range(2):
                        ps = pps.tile([128, 512], F32, tag="sc")
                        oh = pp.tile([128, 2, 512], BF16, tag="oh", bufs=1)
                        nc.sync.dma_start(
                            oh[:],
                            d["onehotT"].ap().rearrange("(a q) t -> q a t", q=128)
                            [:, :, c * 512:(c + 1) * 512])
                        for vk in range(2):
                            nc.tensor.matmul(ps, wemb[:, vk], oh[:, vk],
                                             start=(vk == 0), stop=(vk == 1))
                        nc.scalar.copy(xb[:, c * 512:(c + 1) * 512], ps)
                    ps = pps.tile([128, 512], F32, tag="sc")
                    oho = pp.tile([128, 2, 512], BF16, tag="oh", bufs=1)
                    nc.sync.dma_start(
                        oho[:], d["onehotTo"].ap().rearrange("(a q) t -> q a t", q=128))
                    for vk in range(2):
                        nc.tensor.matmul(ps, wemb[:, vk], oho[:, vk],
                                         start=(vk == 0), stop=(vk == 1))
                    nc.vector.tensor_copy(xo[:], ps)
                    XBall.append(xb)
                    x0own.append(xo)

                # --- stage 1: full layers 0, 1 ---
                XF = x0own
                for k4, li in enumerate((0, 1)):
                    lw = load_layer_full(k4, li, rep)
                    XF, XB, XBn = _layer(nc, pools, lw, XF, XBall, T, T // 2, consts,
                                         do_rs=True, do_ag=(li == 0))
                    if XBn is not None:
                        XBall = XBn

                residF = []
                for m in range(KD):
                    r = pp.tile([128, T // 2], BF16, tag="residF", name=f"res{m}")
                    nc.vector.tensor_copy(r[:], XF[m][:])
                    residF.append(r)

                # --- downsample: pool own token half -> own segment half ---
                XN = []
                for tt in range(4):
                    xn = pp.tile([128, D], BF16, tag="xn", name=f"xn{tt}", bufs=4)
                    for m in range(KD):
                        pt = pps.tile([128, 128], BF16, tag="sc")
                        nc.tensor.transpose(pt[:], XB[m][:, ts(tt)], idbf)
                        nc.vector.tensor_copy(xn[:, ts(m)], pt[:])
                    XN.append(xn)
                wpl = [pp.tile([128, 128], BF16, tag="ohw", name=f"pl{tt}", bufs=4)
                       for tt in range(4)]
                for tt in range(4):
                    nc.sync.dma_start(wpl[tt][:], d["wpool"].ap()[tt])
                sbin = dram.tile([KD, 128, 128], BF16, tag=f"sgi_{rep}")
                sbout = dram.tile([2, KD, 128, 128], BF16, tag=f"sgo_{rep}")
                for m in range(KD):
                    ps = pps.tile([128, 128], F32, tag="sc")
                    for tt in range(4):
                        nc.tensor.matmul(ps, XN[tt][:, ts(m)], wpl[tt][:],
                                         start=(tt == 0), stop=(tt == 3))
                    sb = pp.tile([128, 128], BF16, tag="lnt", name=f"sb{m}")
                    nc.vector.tensor_copy(sb[:], ps)
                    nc.sync.dma_start(sbin[m], sb[:])
                if NO_COLL:
                    nc.sync.dma_start(sbout[0], sbin[:])
                    nc.sync.dma_start(sbout[1], sbin[:])
                else:
                    nc.gpsimd.collective_compute(
                        "AllGather", ALU.bypass, replica_groups=GROUPS,
                        ins=[sbin.opt()], outs=[sbout.opt()])
                nullv = pc.tile([128, KD], F32, tag="c_null")
                nc.sync.dma_start(nullv[:], d["nullv"].ap().rearrange("(a q) -> q a", q=128))
                spre = []
                for m in range(KD):
                    sall = pp.tile([128, SP], BF16, tag="uTs", name=f"sall{m}", bufs=4)
                    nc.gpsimd.memset(sall[:], 0.0)
                    nc.scalar.dma_start(sall[:, 1:129], sbout[0, m])
                    nc.scalar.dma_start(sall[:, 129:257], sbout[1, m])
                    nc.vector.tensor_copy(sall[:, 0:1], nullv[:, m:m + 1])
                    spre.append(sall)
                gdt = pc.tile([128, KD], F32, tag="c_gd")
                nc.sync.dma_start(gdt[:], d["gd"].ap().rearrange("(a q) -> q a", q=128))
                bdt = pc.tile([128, KD], F32, tag="c_bd")
                nc.sync.dma_start(bdt[:], d["bdn"].ap().rearrange("(a q) -> q a", q=128))
                SXF, SXB = _ln(nc, pp, pps, spre, SP, gdt, bdt)

                # --- stage 2: short layers, fully replicated ---
                for k4, li in enumerate(SHORT_LAYERS):
                    lw = load_layer_short(k4, li, rep)
                    SXF, SXB, _ = _layer(nc, pools, lw, SXF, SXB, SP, SP, consts,
                                         do_rs=False, do_ag=False)

                # --- upsample own half + residual ---
                SN = []
                for st in range(SP // 128):
                    sn = pp.tile([128, D], BF16, tag="xn", name=f"sn{st}", bufs=4)
                    for m in range(KD):
                        pt = pps.tile([128, 128], BF16, tag="sc")
                        nc.tensor.transpose(pt[:], SXB[m][:, ts(st)], idbf)
                        nc.vector.tensor_copy(sn[:, ts(m)], pt[:])
                    SN.append(sn)
                uupo = [pp.tile([128, T // 2], BF16, tag="uup", name=f"uu{st}", bufs=3)
                        for st in range(SP // 128)]
                for st in range(SP // 128):
                    nc.sync.dma_start(uupo[st][:], d["uupo"].ap()[ts(st), :])
                XF6, XB6 = [], []
                for m in range(KD):
                    xf = pp.tile([128, T // 2], F32, tag="XF", name=f"XF{m}", bufs=4)
                    xb = pp.tile([128, T // 2], BF16, tag="XB", name=f"XB{m}", bufs=4)
                    for tt in range(4):
                        ps = pps.tile([128, 128], F32, tag="sc")
                        for st in range(SP // 128):
                            nc.tensor.matmul(ps, SN[st][:, ts(m)], uupo[st][:, ts(tt)],
                                             start=(st == 0), stop=(st == SP // 128 - 1))
                        nc.vector.tensor_tensor(xf[:, ts(tt)], ps,
                                                residF[m][:, ts(tt)], ALU.add)
                        nc.scalar.copy(xb[:, ts(tt)], xf[:, ts(tt)])
                    XF6.append(xf)
                    XB6.append(xb)
                XBall = _ag_x(nc, pp, dram, XB6, T, tag=f"up_{rep}")
                XF = XF6

                # --- stage 3: full layers 6, 7 ---
                for k4, li in enumerate((6, 7)):
                    lw = load_layer_full(k4 + 2, li, rep)
                    XF, XB, XBn = _layer(nc, pools, lw, XF, XBall, T, T // 2, consts,
                                         do_rs=True, do_ag=(li == 6))
                    if XBn is not None:
                        XBall = XBn

                # --- final vocab projection (own token half, full V) ---
                finT = [pp.tile([128, V], BF16, tag="vb", name=f"fin{k}", bufs=8)
                        for k in range(KD)]
                for k in range(KD):
                    nc.sync.dma_start(finT[k][:], d["finT"].ap()[ts(k), :])
                fbn = pc.tile([1, V], F32, tag="c_fbn")
                nc.sync.dma_start(fbn[:], d["fbn"].ap()[None, :])
                for tt in range(4):
                    ps = pps.tile([128, 512], F32, tag="sc")[:, :V]
                    for kd in range(KD):
                        nc.tensor.matmul(ps, XB[kd][:, ts(tt)], finT[kd][:],
                                         start=(kd == 0), stop=(kd == KD - 1))
                    nc.tensor.matmul(ps, ones_row, fbn[:], start=False, stop=True)
                    lo = p.tile([128, V], F32, tag="louts", bufs=1)
                    nc.vector.tensor_copy(lo[:], ps)
                    nc.sync.dma_start(logits.ap()[ts(tt), :], lo[:])

    nc.compile()
    return nc


def host_inputs(inputs):
    bf = lambda x: np.ascontiguousarray(np.asarray(x, dtype=np.float32)).astype(ml_dtypes.bfloat16)
    f32 = lambda x: np.ascontiguousarray(np.asarray(x), dtype=np.float32)
    qkv_w = f32(inputs['qkv_w'])
    rk_w = f32(inputs['rk_w'])
    o_w = f32(inputs['o_w'])
    ff_w1 = f32(inputs['ff_w1'])
    ff_w2 = f32(inputs['ff_w2'])
    r_w_bias = f32(inputs['r_w_bias'])
    r_r_bias = f32(inputs['r_r_bias'])
    data = np.asarray(inputs['data'])
    bnd = np.asarray(inputs['boundaries_gt'])

    inv = 1.0 / (10000.0 ** (np.arange(0, D, 2, dtype=np.float32) / D))
    ang = np.arange(T, dtype=np.float32)[:, None] * inv[None, :]
    sin_tab = np.concatenate([np.sin(ang), np.cos(ang)], -1).astype(np.float32)
    eye = np.eye(128, dtype=np.float32)

    in_maps = []
    for c in range(N_CORES):
        b, h = c // 2, c % 2
        heads = list(range(h * 4, h * 4 + 4))
        qr = np.concatenate([np.arange(g * DH, (g + 1) * DH) for g in heads])

        im = {}
        im['wqkvT4'] = bf(np.stack(
            [qkv_w[l][np.concatenate([qr, 512 + qr, 1024 + qr])].T
             for l in FULL_LAYERS]))
        im['wrkT4'] = bf(np.stack([rk_w[l][qr].T for l in FULL_LAYERS]))
        im['woT4'] = bf(np.stack([o_w[l][:, qr].T for l in FULL_LAYERS]))
        im['wqkvS'] = bf(np.stack([qkv_w[l].T for l in SHORT_LAYERS]))
        im['wrkS'] = bf(np.stack([rk_w[l].T for l in SHORT_LAYERS]))
        im['woS'] = bf(np.stack([o_w[l].T for l in SHORT_LAYERS]))
        im['w1T'] = bf(np.stack([ff_w1[l].T for l in range(L)]))
        im['w2T'] = bf(np.stack([ff_w2[l].T for l in range(L)]))
        im['fb1'] = f32(inputs['ff_b1'])
        im['fb2'] = f32(inputs['ff_b2'])
        im['g1'] = f32(inputs['ln1_g'])
        im['bb1'] = f32(inputs['ln1_b'])
        im['g2'] = f32(inputs['ln2_g'])
        im['bb2'] = f32(inputs['ln2_b'])
        im['rwb4'] = f32(r_w_bias[heads].reshape(-1))
        im['rrb4'] = f32(r_r_bias[heads].reshape(-1))
        im['rwbS'] = f32(r_w_bias.reshape(-1))
        im['rrbS'] = f32(r_r_bias.reshape(-1))
        im['wemb'] = bf(inputs['word_emb'])
        oh = np.zeros((V, T), np.float32)
        oh[data[:, b], np.arange(T)] = 1.0
        im['onehotT'] = bf(oh)
        im['onehotTo'] = bf(oh[:, h * 512:(h + 1) * 512])
        im['sinTd'] = bf(sin_tab.T)
        im['idbf'] = bf(eye)
        hard = bnd[:, b].astype(np.float32)
        seg = np.cumsum(hard) - hard
        ind = (seg[:, None] == np.arange(S)).astype(np.float32)
        wmat = ind / (ind.sum(0, keepdims=True) + 1e-9)       # (T, S)
        im['wpool'] = bf(np.stack(
            [wmat[(h * 4 + tt) * 128:(h * 4 + tt + 1) * 128,
                  h * 128:(h + 1) * 128] for tt in range(4)]))
        im['nullv'] = f32(np.asarray(inputs['null_group']).reshape(-1))
        im['gd'] = f32(inputs['down_ln_g'])
        im['bdn'] = f32(inputs['down_ln_b'])
        segU = np.clip(np.cumsum(hard).astype(np.int64), 0, S)
        uu = np.zeros((SP, T), np.float32)
        uu[segU, np.arange(T)] = 1.0
        im['uupo'] = bf(uu[:, h * 512:(h + 1) * 512])
        im['finT'] = bf(f32(inputs['final_w']).T)
        im['fbn'] = f32(np.asarray(inputs['final_b']))
        in_maps.append(im)
    return in_maps


_NC_CACHE = {}


def get_program():
    if 'nc' not in _NC_CACHE:
        _NC_CACHE['nc'] = build_program()
    return _NC_CACHE['nc']


def kernel(**inputs) -> np.ndarray:
    nc = get_program()
    in_maps = host_inputs(inputs)
    res = run_bass_kernel_spmd(nc, in_maps, core_ids=list(range(N_CORES)), trace=False)
    out = np.zeros((T, B, V), np.float32)
    for c in range(N_CORES):
        b, h = c // 2, c % 2
        out[h * 512:(h + 1) * 512, b, :] = res.results[c]['logits']
    return out
